# revision 1
# baseline (speedup 1.0000x reference)
"""Masked (expander) linear layer on 8 Trainium2 NeuronCores.

Computes out = x @ (W * M)^T for
  x: [16384, 2048] f32, W: [2048, 2048] f32, M: [2048, 2048] int32 (0/1)

Sharding: pure data-parallel over rows of x. Each of the 8 cores gets 2048
rows of x plus a replicated (transposed) copy of W and M, computes its
[2048, 2048] output shard entirely locally (mask-multiply on DVE, matmul on
PE), and the host concatenates shards. No collectives.

Device-side design:
 - All tensors are laid out on host so the contraction dim lands on SBUF
   partitions: W and M transposed panel-major ([NT, IN, 512], contiguous
   panels), x transposed per core ([IN, rows]). Layout-only host prep;
   every FLOP of the module (mask multiply + matmul) runs on device. The
   mask is passed as int8 (0/1, lossless repack) to cut DMA traffic.
 - Matmuls run in float32r mode (single-pass fp32_mode=HIGH PE streaming,
   1 cycle/row, vs 4 cycles/row for plain fp32; 1.35e-4 rel err at
   K=2048). The walrus verifier requires f32r operands to come from
   f32r-rounding producers: the DVE mask-multiply writes wm as f32r and
   x streams in through SWDGE cast-DMAs (f32 -> f32r).
 - Each DMA ring carries one stream so per-DMA fixed costs overlap:
   W 1MB k-quarter pieces on the sync HWDGE ring (3-deep staging
   pipeline), masks on the scalar ring, x on the SWDGE ring, outputs on
   the scalar ring. wm is stored as one tile per (n-chunk, k-quarter) so
   matmul sub-groups depend only on their own piece - PE starts ~17us in
   and stays fed through the whole weight load (keeps HAM at full clock).
 - m-tiles are processed in blocks of 4, n-chunk outer inside a block;
   x quarter-tiles are single-buffered and re-streamed just-in-time (the
   next block's quarter q loads right behind this block's last reader of
   quarter q). PSUM groups rotate over 8 banks with evacuation (ScalarE
   copy + DMA) inlined right after each group closes. (PSUM groups
   rotate over all 8 banks.)
"""

from contextlib import ExitStack

import numpy as np

import concourse.bacc as bacc
import concourse.bass as bass
import concourse.mybir as mybir
import concourse.tile as tile
from concourse.bass_utils import run_bass_kernel_spmd

N_CORES = 8
P = 128

FULL_N, FULL_OUT, FULL_IN = 16384, 2048, 2048

MASK_DTYPES = {
    "int8": (mybir.dt.int8, np.int8),
    "int32": (mybir.dt.int32, np.int32),
    "float32": (mybir.dt.float32, np.float32),
}


def build_nc(
    rows: int = FULL_N // N_CORES,
    in_dim: int = FULL_IN,
    out_dim: int = FULL_OUT,
    mm_dtype=mybir.dt.float32r,
    mask_dtype: str = "int8",
    n_chunk: int = 512,
    m_block: int = 4,
):
    """Per-core Bass module: y[rows, out] = x @ (wt * m).

    DRAM layouts: wt/mk panel-major [NT, in_dim, n_chunk]; x transposed
    [in_dim, rows]; y row-major [rows, out_dim].
    """
    assert rows % P == 0 and in_dim % P == 0 and out_dim % n_chunk == 0
    KT = in_dim // P
    MT = rows // P
    NT = out_dim // n_chunk
    assert KT % 4 == 0 and MT % m_block == 0
    KQ = KT // 4
    NB = MT // m_block
    mw = m_block * P  # columns of x per block

    mdt, _ = MASK_DTYPES[mask_dtype]

    nc = bacc.Bacc("TRN2", target_bir_lowering=False, debug=False)
    x = nc.dram_tensor("x", [in_dim, rows], mybir.dt.float32, kind="ExternalInput")
    wt = nc.dram_tensor(
        "wt", [NT, in_dim, n_chunk], mybir.dt.float32, kind="ExternalInput"
    )
    mk = nc.dram_tensor("mk", [NT, in_dim, n_chunk], mdt, kind="ExternalInput")
    y = nc.dram_tensor("y", [rows, out_dim], mybir.dt.float32, kind="ExternalOutput")

    # K-major DRAM views: [.., p, kt, ..]
    wt_v = wt[:, :, :].rearrange("t (kt p) n -> t p kt n", p=P)
    mk_v = mk[:, :, :].rearrange("t (kt p) n -> t p kt n", p=P)
    x_v = x[:, :].rearrange("(kt p) m -> p kt m", p=P)

    with ExitStack() as ctx:
        tc = ctx.enter_context(tile.TileContext(nc))
        wm_pool = ctx.enter_context(tc.tile_pool(name="wm", bufs=1))
        ws_pool = ctx.enter_context(tc.tile_pool(name="ws", bufs=3))
        msk_pool = ctx.enter_context(tc.tile_pool(name="msk", bufs=3))
        xt_pool = ctx.enter_context(tc.tile_pool(name="xt", bufs=1))
        yo_pool = ctx.enter_context(tc.tile_pool(name="yo", bufs=3))
        pm_pool = ctx.enter_context(tc.tile_pool(name="pm", bufs=1, space="PSUM"))

        # Resident masked weight: wm_t[nt][q] of shape [P, KQ, n_chunk]
        wm_t = [
            [
                wm_pool.tile(
                    [P, KQ, n_chunk], mm_dtype, tag=f"wm{nt}_{q}", name=f"wm{nt}_{q}"
                )
                for q in range(4)
            ]
            for nt in range(NT)
        ]
        # x tiles: [P, KQ, m_block*P] per k-quarter (single set; the next
        # block's quarter q streams in right after this block's last reader
        # of quarter q)
        xt_t = [
            xt_pool.tile([P, KQ, mw], mm_dtype, tag=f"xt{q}", name=f"xt{q}")
            for q in range(4)
        ]

        def load_w_piece(nt, q):
            ksl = slice(q * KQ, (q + 1) * KQ)
            # W rides the sync HWDGE ring alone (own per-DMA fixed costs)
            wstage = ws_pool.tile([P, KQ, n_chunk], mybir.dt.float32, tag="ws")
            nc.sync.dma_start(out=wstage[:], in_=wt_v[nt, :, ksl, :])
            # masks ride the scalar ring (done before output stores begin)
            mtile = msk_pool.tile([P, KQ, n_chunk], mdt, tag="mt")
            nc.scalar.dma_start(out=mtile[:], in_=mk_v[nt, :, ksl, :])
            for k in range(KQ):
                # masked multiply; DVE f32r output is the rounding producer
                nc.vector.tensor_mul(
                    wm_t[nt][q][:, k, :], wstage[:, k, :], mtile[:, k, :]
                )

        def load_x_piece(b, q):
            ksl = slice(q * KQ, (q + 1) * KQ)
            # SWDGE cast-DMA f32 -> f32r (the rounding producer); x has the
            # SWDGE ring to itself. Two m-half DMAs: the first half's WAR
            # clears as soon as mb 0/1 finish reading, so the JIT re-stream
            # at block boundaries starts (and lands) earlier.
            hw = mw // 2
            for h in range(2):
                nc.gpsimd.dma_start(
                    out=xt_t[q][:, :, h * hw : (h + 1) * hw],
                    in_=x_v[:, ksl, b * mw + h * hw : b * mw + (h + 1) * hw],
                )

        # ---- prep: x block 0 on the SWDGE ring, W pieces on sync ----
        for q in range(4):
            load_x_piece(0, q)
        for nt in range(NT):
            for q in range(4):
                load_w_piece(nt, q)

        # ---- main: blocks of m_block m-tiles; nt-outer inside a block ----
        for b in range(NB):
            xts = xt_t
            for nt in range(NT):
                # 6 rotating PSUM banks: group g frees its bank 6 groups later
                pms = {
                    mb: pm_pool.tile(
                        [P, n_chunk],
                        mybir.dt.float32,
                        tag=f"pm{(nt * m_block + mb) % 8}",
                        name=f"pm{(nt * m_block + mb) % 8}",
                    )
                    for mb in range(m_block)
                }
                # k-quarter-outer: each sub-group only needs its own pieces
                for q in range(4):
                    for mb in range(m_block):
                        for k in range(KQ):
                            kt = q * KQ + k
                            nc.tensor.matmul(
                                pms[mb][:],
                                xts[q][:, k, bass.ts(mb, P)],
                                wm_t[nt][q][:, k, :],
                                start=(kt == 0),
                                stop=(kt == KT - 1),
                            )
                        if q == 3:
                            # evacuate as soon as this group closes
                            mt = b * m_block + mb
                            yo = yo_pool.tile(
                                [P, n_chunk], mybir.dt.float32, tag="yo"
                            )
                            nc.scalar.copy(yo[:], pms[mb][:])
                            nc.scalar.dma_start(
                                out=y[mt * P : (mt + 1) * P, bass.ts(nt, n_chunk)],
                                in_=yo[:],
                            )
                    if nt == NT - 1 and b + 1 < NB:
                        # last reader of x quarter q just finished; stream in
                        # the next block's quarter q behind it
                        load_x_piece(b + 1, q)

    nc.compile()
    return nc


def _prep_host(input_, weight, mask, mask_dtype="int8", n_chunk=512):
    _, npdt = MASK_DTYPES[mask_dtype]
    in_dim, out_dim = weight.shape[1], weight.shape[0]
    nt = out_dim // n_chunk
    # weight.T -> [NT, IN, n_chunk], each panel contiguous
    wtp = np.ascontiguousarray(weight.T.reshape(in_dim, nt, n_chunk).transpose(1, 0, 2))
    mkp = np.ascontiguousarray(
        mask.T.reshape(in_dim, nt, n_chunk).transpose(1, 0, 2)
    ).astype(npdt)
    rows = input_.shape[0] // N_CORES
    in_maps = []
    for c in range(N_CORES):
        xp = np.ascontiguousarray(input_[c * rows : (c + 1) * rows].T)
        in_maps.append({"x": xp, "wt": wtp, "mk": mkp})
    return in_maps


_CACHE = {}


def _run(input_, weight, mask, trace=False, **build_kw):
    rows_total, in_dim = input_.shape
    out_dim = weight.shape[0]
    key = (rows_total, in_dim, out_dim, tuple(sorted(build_kw.items())))
    if key not in _CACHE:
        _CACHE[key] = build_nc(
            rows=rows_total // N_CORES, in_dim=in_dim, out_dim=out_dim, **build_kw
        )
    nc = _CACHE[key]
    in_maps = _prep_host(
        input_,
        weight,
        mask,
        build_kw.get("mask_dtype", "int8"),
        build_kw.get("n_chunk", 512),
    )
    res = run_bass_kernel_spmd(nc, in_maps, core_ids=list(range(N_CORES)), trace=trace)
    out = np.concatenate([res.results[c]["y"] for c in range(N_CORES)], axis=0)
    return out, res


def kernel(input_, weight, mask):
    input_ = np.asarray(input_, dtype=np.float32)
    weight = np.asarray(weight, dtype=np.float32)
    mask = np.asarray(mask)
    out, _ = _run(input_, weight, mask, trace=False)
    return out



# revision 4
# speedup vs baseline: 1.0769x; 1.0769x over previous
"""Masked (expander) linear layer on 8 Trainium2 NeuronCores.

Computes out = x @ (W * M)^T for
  x: [16384, 2048] f32, W: [2048, 2048] f32, M: [2048, 2048] int32 (0/1)

Sharding: pure data-parallel over rows of x. Each of the 8 cores gets 2048
rows of x plus a replicated copy of W and M, computes its [2048, 2048]
output shard (transposed) locally, and the host transposes + concatenates.
No collectives.

Device-side design (v2):
 - Operands in bf16 (host casts x and W; mask as int8). bf16 matmul runs
   at the same 1 cycle/row PE rate as f32r but halves all input DMA
   traffic, and bf16 weights are mandatory for stationary-reuse (walrus
   cannot pair ldweights with 4-byte dtypes). Accumulation stays f32 in
   PSUM; rel err ~2e-3 vs the f32 reference.
 - Orientation: y^T = (W*M) @ x^T. Stationary operand = [128,128] piece
   of the masked weight, moving operand = 512-row chunk of x^T. Each
   stationary piece feeds 4 consecutive matmuls (the 4 row-chunks), so
   the per-matmul LD_WEIGHTS (128 PE cycles, serialized on TRN2) is
   amortized 4x when codegen dedups the repeated stationary AP.
 - x^T is fully resident in SBUF (8.4 MB bf16), streamed in 16 k-slices
   on its own ring so the first panel sweep can track the stream. W and
   masks stream panel-by-panel (one 512-col panel per ~52us of PE work);
   the DVE applies the mask (bf16*int8->bf16) in [128,4,512] pieces.
 - Each 512-col W panel is processed as two half-sweeps of 2 sub-tiles,
   k-major: 8 PSUM groups live per half (all 8 banks), which paces
   panel-0 PE demand at ~1.6us per x k-slice, just behind the ~1.5us
   per-slice HBM supply. PSUM evacuation alternates between the Scalar
   and Vector engines so the 8 simultaneous group-closes at a half
   boundary drain at 2x rate while the PE starts the next half.
"""

from contextlib import ExitStack

import ml_dtypes
import numpy as np

import concourse.bacc as bacc
import concourse.bass as bass
import concourse.mybir as mybir
import concourse.tile as tile
from concourse.bass_utils import run_bass_kernel_spmd

N_CORES = 8
P = 128

FULL_N, FULL_OUT, FULL_IN = 16384, 2048, 2048


def build_nc(
    rows: int = FULL_N // N_CORES,
    in_dim: int = FULL_IN,
    out_dim: int = FULL_OUT,
    n_panel: int = 512,
):
    """Per-core Bass module: yt[out, rows] = (wt * m)^T-contracted with x.

    DRAM layouts: x transposed [in_dim, rows] bf16; wt/mk panel-major
    [NTP, in_dim, n_panel] (bf16 / int8); yt [out_dim, rows] f32.
    """
    assert rows % 512 == 0 and in_dim % P == 0 and out_dim % n_panel == 0
    KT = in_dim // P  # 16 k-tiles
    NTP = out_dim // n_panel  # 4 weight panels
    SUBS = n_panel // P  # 4 stationary sub-tiles per panel
    MC = rows // 512  # 4 moving row-chunks
    KQ = 4  # k-tiles per W/mask DMA piece
    NKQ = KT // KQ

    bf16 = mybir.dt.bfloat16

    nc = bacc.Bacc("TRN2", target_bir_lowering=False, debug=False)
    x = nc.dram_tensor("x", [in_dim, rows], bf16, kind="ExternalInput")
    wt = nc.dram_tensor("wt", [NTP, in_dim, n_panel], bf16, kind="ExternalInput")
    mk = nc.dram_tensor("mk", [NTP, in_dim, n_panel], mybir.dt.int8, kind="ExternalInput")
    yt = nc.dram_tensor("yt", [out_dim, rows], mybir.dt.float32, kind="ExternalOutput")

    # K-major views: [.., p, kt, ..]
    wt_v = wt[:, :, :].rearrange("t (kt p) n -> t p kt n", p=P)
    mk_v = mk[:, :, :].rearrange("t (kt p) n -> t p kt n", p=P)
    x_v = x[:, :].rearrange("(kt p) m -> p kt m", p=P)

    with ExitStack() as ctx:
        tc = ctx.enter_context(tile.TileContext(nc))
        xt_pool = ctx.enter_context(tc.tile_pool(name="xt", bufs=1))
        wm_pool = ctx.enter_context(tc.tile_pool(name="wm", bufs=1))
        ws_pool = ctx.enter_context(tc.tile_pool(name="ws", bufs=3))
        msk_pool = ctx.enter_context(tc.tile_pool(name="msk", bufs=3))
        yo_pool = ctx.enter_context(tc.tile_pool(name="yo", bufs=4))
        pm_pool = ctx.enter_context(tc.tile_pool(name="pm", bufs=1, space="PSUM"))

        # Resident x^T: [P, KT, rows] bf16, filled by 16 k-slice DMAs in
        # order on the SWDGE ring so panel 0 can track the stream.
        xt = xt_pool.tile([P, KT, rows], bf16, tag="xt", name="xt")
        for k in range(KT):
            nc.gpsimd.dma_start(out=xt[:, k : k + 1, :], in_=x_v[:, k : k + 1, :])

        # Masked-weight tiles: one per (panel, kq), [P, KQ, n_panel] bf16.
        # Current panel + next panel in flight -> 2*NKQ buffered tags.
        wm_t = [
            [
                wm_pool.tile(
                    [P, KQ, n_panel], bf16, tag=f"wm{t % 2}_{q}", name=f"wm{t}_{q}"
                )
                for q in range(NKQ)
            ]
            for t in range(NTP)
        ]

        def load_w_piece(t, q):
            ksl = slice(q * KQ, (q + 1) * KQ)
            wstage = ws_pool.tile([P, KQ, n_panel], bf16, tag="ws")
            nc.sync.dma_start(out=wstage[:], in_=wt_v[t, :, ksl, :])
            mtile = msk_pool.tile([P, KQ, n_panel], mybir.dt.int8, tag="mt")
            nc.scalar.dma_start(out=mtile[:], in_=mk_v[t, :, ksl, :])
            nc.vector.tensor_mul(wm_t[t][q][:], wstage[:], mtile[:])

        # Prime: panel 0 and 1 weight pieces. Panel t+1 (t>=1) loads at the
        # start of panel t: its wm tag conflicts only with panel t-1, whose
        # readers are already retiring, so the DVE mult never blocks the
        # evac copies queued behind it (no PSUM deadlock cycle).
        for t in range(2):
            for q in range(NKQ):
                load_w_piece(t, q)

        evac_n = 0

        def evac(pm, t, sub, mc):
            nonlocal evac_n
            yo = yo_pool.tile([P, 512], mybir.dt.float32, tag="yo")
            if evac_n % 2 == 0:
                nc.scalar.copy(yo[:], pm[:])
            else:
                nc.vector.tensor_copy(yo[:], pm[:])
            evac_n += 1
            nc.scalar.dma_start(
                out=yt[(t * SUBS + sub) * P : (t * SUBS + sub + 1) * P, bass.ts(mc, 512)],
                in_=yo[:],
            )

        for t in range(NTP):
            if 1 <= t and t + 1 <= NTP - 1:
                for q in range(NKQ):
                    load_w_piece(t + 1, q)
            for half in range(2):
                subs = (2 * half, 2 * half + 1)
                # 8 live PSUM groups: (sub, mc) -> bank (sub%2)*4 + mc
                pms = {
                    (sub, mc): pm_pool.tile(
                        [P, 512],
                        mybir.dt.float32,
                        tag=f"pm{(sub % 2) * 4 + mc}",
                        name=f"pm{(sub % 2) * 4 + mc}",
                    )
                    for sub in subs
                    for mc in range(MC)
                }
                for k in range(KT):
                    q, kk = k // KQ, k % KQ
                    for sub in subs:
                        stat = wm_t[t][q][:, kk, sub * P : (sub + 1) * P]
                        for mc in range(MC):
                            nc.tensor.matmul(
                                pms[(sub, mc)][:],
                                stat,
                                xt[:, k, bass.ts(mc, 512)],
                                start=(k == 0),
                                stop=(k == KT - 1),
                            )
                # groups closed; drain in the order the next half reacquires
                for sub in subs:
                    for mc in range(MC):
                        evac(pms[(sub, mc)], t, sub, mc)

    nc.compile()
    return nc


def _prep_host(input_, weight, mask, n_panel=512):
    in_dim, out_dim = weight.shape[1], weight.shape[0]
    ntp = out_dim // n_panel
    wtp = np.ascontiguousarray(
        weight.T.reshape(in_dim, ntp, n_panel).transpose(1, 0, 2)
    ).astype(ml_dtypes.bfloat16)
    mkp = np.ascontiguousarray(
        mask.T.reshape(in_dim, ntp, n_panel).transpose(1, 0, 2)
    ).astype(np.int8)
    rows = input_.shape[0] // N_CORES
    in_maps = []
    for c in range(N_CORES):
        xp = np.ascontiguousarray(input_[c * rows : (c + 1) * rows].T).astype(
            ml_dtypes.bfloat16
        )
        in_maps.append({"x": xp, "wt": wtp, "mk": mkp})
    return in_maps


_CACHE = {}


def _run(input_, weight, mask, trace=False, **build_kw):
    rows_total, in_dim = input_.shape
    out_dim = weight.shape[0]
    key = (rows_total, in_dim, out_dim, tuple(sorted(build_kw.items())))
    if key not in _CACHE:
        _CACHE[key] = build_nc(
            rows=rows_total // N_CORES, in_dim=in_dim, out_dim=out_dim, **build_kw
        )
    nc = _CACHE[key]
    in_maps = _prep_host(input_, weight, mask, build_kw.get("n_panel", 512))
    res = run_bass_kernel_spmd(nc, in_maps, core_ids=list(range(N_CORES)), trace=trace)
    out = np.concatenate(
        [np.ascontiguousarray(res.results[c]["yt"].T) for c in range(N_CORES)], axis=0
    )
    return out, res


def kernel(input_, weight, mask):
    input_ = np.asarray(input_, dtype=np.float32)
    weight = np.asarray(weight, dtype=np.float32)
    mask = np.asarray(mask)
    out, _ = _run(input_, weight, mask, trace=False)
    return out


# revision 6
# speedup vs baseline: 1.0775x; 1.0005x over previous
"""Masked (expander) linear layer on 8 Trainium2 NeuronCores.

Computes out = x @ (W * M)^T for
  x: [16384, 2048] f32, W: [2048, 2048] f32, M: [2048, 2048] int32 (0/1)

Sharding: pure data-parallel over rows of x. Each of the 8 cores gets 2048
rows of x plus a replicated copy of W and M, computes its [2048, 2048]
output shard (transposed) locally, and the host transposes + concatenates.
No collectives.

Device-side design (v3):
 - Operands in bf16 (host casts x and W; mask as int8). bf16 matmul
   streams at 1 row/cycle like f32r, but its 2-byte LD_WEIGHTS fully
   hides under the previous matmul's 512-row stream (f32r's 4-byte load
   does not), so the steady-state matmul period is the 216ns floor.
   Accumulation stays f32 in PSUM; rel err ~2e-3 vs the f32 reference.
 - Orientation: y^T = (W*M) @ x^T. Stationary operand = [128,128] piece
   of the masked weight, moving operand = 512-row chunk of x^T; a
   [128,512] PSUM group accumulates over the 16 k-tiles.
 - x^T is fully resident in SBUF (8.4 MB bf16). DMA issue order is the
   consumption order: W piece 0 + mask 0 first, then x k-slices
   interleaved with the remaining panel-0 pieces, then the x tail, then
   panel 1 — the 16 shared DMA queues drain roughly in issue order, so
   the first masked-weight piece lands ~5us in instead of queueing
   behind all of x. Panel t+1 streams during panel t (t>=1).
 - While the first W piece is in flight, ~12 warm-up matmuls on a
   memset scratch tile keep the PE busy from ~1us so the clock p-state
   is fully ramped before real work starts.
 - Panel 0's first half runs k-major over sub-tiles 0,1 (8 live PSUM
   groups) so PE demand tracks the in-flight x stream; everything after
   runs sub-major (4 live groups), which staggers group-closes and makes
   PSUM bank handoffs and the final drain cheap. Evacuation alternates
   ScalarE/DVE in bank-reacquisition order; y stores alternate the
   scalar and sync DMA rings.
"""

from contextlib import ExitStack

import ml_dtypes
import numpy as np

import concourse.bacc as bacc
import concourse.bass as bass
import concourse.mybir as mybir
import concourse.tile as tile
from concourse.bass_utils import run_bass_kernel_spmd

N_CORES = 8
P = 128

FULL_N, FULL_OUT, FULL_IN = 16384, 2048, 2048


def build_nc(
    rows: int = FULL_N // N_CORES,
    in_dim: int = FULL_IN,
    out_dim: int = FULL_OUT,
    n_panel: int = 512,
    warm_mms: int = 12,
):
    """Per-core Bass module: yt[out, rows] = (wt * m) contracted with x.

    DRAM layouts: x transposed [in_dim, rows] bf16; wt/mk panel-major
    [NTP, in_dim, n_panel] (bf16 / int8); yt [out_dim, rows] f32.
    """
    assert rows % 512 == 0 and in_dim % P == 0 and out_dim % n_panel == 0
    KT = in_dim // P  # 16 k-tiles
    NTP = out_dim // n_panel  # 4 weight panels
    SUBS = n_panel // P  # 4 stationary sub-tiles per panel
    MC = rows // 512  # 4 moving row-chunks
    KQ = 4  # k-tiles per W/mask DMA piece
    NKQ = KT // KQ

    bf16 = mybir.dt.bfloat16

    nc = bacc.Bacc("TRN2", target_bir_lowering=False, debug=False)
    x = nc.dram_tensor("x", [in_dim, rows], bf16, kind="ExternalInput")
    wt = nc.dram_tensor("wt", [NTP, in_dim, n_panel], bf16, kind="ExternalInput")
    mk = nc.dram_tensor("mk", [NTP, in_dim, n_panel], mybir.dt.int8, kind="ExternalInput")
    yt = nc.dram_tensor("yt", [out_dim, rows], mybir.dt.float32, kind="ExternalOutput")

    # K-major views: [.., p, kt, ..]
    wt_v = wt[:, :, :].rearrange("t (kt p) n -> t p kt n", p=P)
    mk_v = mk[:, :, :].rearrange("t (kt p) n -> t p kt n", p=P)
    x_v = x[:, :].rearrange("(kt p) m -> p kt m", p=P)

    with ExitStack() as ctx:
        tc = ctx.enter_context(tile.TileContext(nc))
        xt_pool = ctx.enter_context(tc.tile_pool(name="xt", bufs=1))
        wm_pool = ctx.enter_context(tc.tile_pool(name="wm", bufs=1))
        ws_pool = ctx.enter_context(tc.tile_pool(name="ws", bufs=3))
        msk_pool = ctx.enter_context(tc.tile_pool(name="msk", bufs=3))
        yo_pool = ctx.enter_context(tc.tile_pool(name="yo", bufs=4))
        wrm_pool = ctx.enter_context(tc.tile_pool(name="wrm", bufs=1))
        pm_pool = ctx.enter_context(tc.tile_pool(name="pm", bufs=1, space="PSUM"))

        # Resident x^T: [P, KT, rows] bf16, k-slice DMAs on the SWDGE ring.
        xt = xt_pool.tile([P, KT, rows], bf16, tag="xt", name="xt")

        def load_x_slice(k):
            nc.gpsimd.dma_start(out=xt[:, k : k + 1, :], in_=x_v[:, k : k + 1, :])

        # Masked-weight tiles: one per (panel, kq), [P, KQ, n_panel] bf16,
        # double-buffered across panels via the tag's t%2.
        wm_t = [
            [
                wm_pool.tile(
                    [P, KQ, n_panel], bf16, tag=f"wm{t % 2}_{q}", name=f"wm{t}_{q}"
                )
                for q in range(NKQ)
            ]
            for t in range(NTP)
        ]

        def load_w_piece(t, q):
            ksl = slice(q * KQ, (q + 1) * KQ)
            wstage = ws_pool.tile([P, KQ, n_panel], bf16, tag="ws")
            nc.sync.dma_start(out=wstage[:], in_=wt_v[t, :, ksl, :])
            mtile = msk_pool.tile([P, KQ, n_panel], mybir.dt.int8, tag="mt")
            nc.scalar.dma_start(out=mtile[:], in_=mk_v[t, :, ksl, :])
            nc.vector.tensor_mul(wm_t[t][q][:], wstage[:], mtile[:])

        # DMA issue order == consumption order. Panel-0 pieces lead and
        # interleave with the x slices their k-window needs.
        for q in range(NKQ):
            load_w_piece(0, q)
            load_x_slice(q)
        for k in range(NKQ, KT):
            load_x_slice(k)
        for q in range(NKQ):
            load_w_piece(1, q)

        # Warm-up: ramp the PE p-state while the first wm piece is in
        # flight. Scratch bf16 moving tile (memset on DVE), self-contained
        # one-shot groups into bank 7 (the last bank real work reacquires).
        warm = wrm_pool.tile([P, 512], bf16, tag="warm", name="warm")
        nc.vector.memset(warm[:], 0.0)
        wpm = pm_pool.tile([P, 512], mybir.dt.float32, tag="pm7", name="pmw")
        for _ in range(warm_mms):
            nc.tensor.matmul(wpm[:], warm[:, :P], warm[:], start=True, stop=True)

        evac_n = 0

        def evac(pm, t, sub, mc):
            nonlocal evac_n
            yo = yo_pool.tile([P, 512], mybir.dt.float32, tag="yo")
            if mc % 2 == 0:
                nc.scalar.copy(yo[:], pm[:])
            else:
                nc.vector.tensor_copy(yo[:], pm[:])
            if evac_n % 2 == 0:
                ydma = nc.scalar.dma_start
            else:
                ydma = nc.sync.dma_start
            evac_n += 1
            ydma(
                out=yt[(t * SUBS + sub) * P : (t * SUBS + sub + 1) * P, bass.ts(mc, 512)],
                in_=yo[:],
            )

        def pm_tile(sub, mc):
            bank = (sub % 2) * 4 + mc
            return pm_pool.tile(
                [P, 512], mybir.dt.float32, tag=f"pm{bank}", name=f"pm{bank}"
            )

        def mm(pms, t, sub, mc, k):
            q, kk = k // KQ, k % KQ
            nc.tensor.matmul(
                pms[(sub, mc)][:],
                wm_t[t][q][:, kk, sub * P : (sub + 1) * P],
                xt[:, k, bass.ts(mc, 512)],
                start=(k == 0),
                stop=(k == KT - 1),
            )

        def sub_sweep(t, sub):
            pms = {(sub, mc): pm_tile(sub, mc) for mc in range(MC)}
            for k in range(KT):
                for mc in range(MC):
                    mm(pms, t, sub, mc, k)
            for mc in range(MC):
                evac(pms[(sub, mc)], t, sub, mc)

        # Panel 0 first half: k-major over subs 0,1 (8 live groups) to
        # track the in-flight x stream; then sub-major everywhere.
        pms0 = {(sub, mc): pm_tile(sub, mc) for sub in (0, 1) for mc in range(MC)}
        for k in range(KT):
            for sub in (0, 1):
                for mc in range(MC):
                    mm(pms0, 0, sub, mc, k)
        for sub in (0, 1):
            for mc in range(MC):
                evac(pms0[(sub, mc)], 0, sub, mc)
        sub_sweep(0, 2)
        sub_sweep(0, 3)

        for t in range(1, NTP):
            if t + 1 <= NTP - 1:
                for q in range(NKQ):
                    load_w_piece(t + 1, q)
            for sub in range(SUBS):
                sub_sweep(t, sub)

    nc.compile()
    return nc


def _prep_host(input_, weight, mask, n_panel=512):
    in_dim, out_dim = weight.shape[1], weight.shape[0]
    ntp = out_dim // n_panel
    wtp = np.ascontiguousarray(
        weight.T.reshape(in_dim, ntp, n_panel).transpose(1, 0, 2)
    ).astype(ml_dtypes.bfloat16)
    mkp = np.ascontiguousarray(
        mask.T.reshape(in_dim, ntp, n_panel).transpose(1, 0, 2)
    ).astype(np.int8)
    rows = input_.shape[0] // N_CORES
    in_maps = []
    for c in range(N_CORES):
        xp = np.ascontiguousarray(input_[c * rows : (c + 1) * rows].T).astype(
            ml_dtypes.bfloat16
        )
        in_maps.append({"x": xp, "wt": wtp, "mk": mkp})
    return in_maps


_CACHE = {}


def _run(input_, weight, mask, trace=False, **build_kw):
    rows_total, in_dim = input_.shape
    out_dim = weight.shape[0]
    key = (rows_total, in_dim, out_dim, tuple(sorted(build_kw.items())))
    if key not in _CACHE:
        _CACHE[key] = build_nc(
            rows=rows_total // N_CORES, in_dim=in_dim, out_dim=out_dim, **build_kw
        )
    nc = _CACHE[key]
    in_maps = _prep_host(input_, weight, mask, build_kw.get("n_panel", 512))
    res = run_bass_kernel_spmd(nc, in_maps, core_ids=list(range(N_CORES)), trace=trace)
    out = np.concatenate(
        [np.ascontiguousarray(res.results[c]["yt"].T) for c in range(N_CORES)], axis=0
    )
    return out, res


def kernel(input_, weight, mask):
    input_ = np.asarray(input_, dtype=np.float32)
    weight = np.asarray(weight, dtype=np.float32)
    mask = np.asarray(mask)
    out, _ = _run(input_, weight, mask, trace=False)
    return out
